# revision 1
# baseline (speedup 1.0000x reference)
"""Multi-head attention (16 heads, N=2048, D=1024, E=64) on 8 Trainium2 cores.

Head-parallel sharding: core m handles heads (2m, 2m+1), computes its two
heads' attention contexts and a partial o_proj (rows 128m:128m+128 of the
row-sharded o_proj); the host sums the 8 partial fp32 outputs in fp64.

All layouts are chosen so no large on-device transposes are needed, and
all matmuls run at the full float32r PE rate (1 cycle/row) while keeping
fp32-level accuracy on the precision-critical softmax path:

  inputs: x^T and the qkv weights arrive hi/lo-split into float32r halves
    (host RNE-11-bit rounding matches the hardware's float32r operand
    rounding exactly, verified on device; 11+11-bit operands multiply
    exactly, so hi@hi + lo@hi + hi@lo is fp32-accurate)
  projections: qT/kT/vT [E, N] = w^T x^T, d-contraction on PE, both heads
    per matmul (their weight columns are concatenated)
  max-pass: S[q,m] score tiles from the hi parts only (error of a few
    units is fine — softmax shift-invariance only needs the shift within
    ~80 of the true row max), DVE free-dim reduce_max -> c_q,
    PE-transposed and DMA-reshaped into qT_ext row 64 as -c_q
  scores: S'^T[m,q] = sum_{e<64} k[m,e]q[q,e] - c_q, via e-extension
    (kT_ext row 64 = 1, qT_ext row 64 = -c_q) in two matmuls per tile:
    one stacked K=128 cross-term matmul [kl;kh]@[qh;ql] + one K=65
    kh_ext@qh_ext carrying the max subtraction
  E^T = exp(S'^T / 8) (ScalarE, straight from PSUM)
  ctx^T/Z: lhsT = v_ext [m, 65] (v columns + a ones column) ->
    psum rows 0:63 = ctx^T, row 64 = Z (the softmax denominator),
    accumulated over the 16 m-blocks
  normalize: 1/Z (DVE) broadcast across partitions (GpSimd) * ctx^T (DVE)
  out_partial[n, :] = ctx_norm_bothheads^T.T @ wo_rows (one K=128 matmul
    per 128-row output block)

The phases are software-pipelined per 512-wide q-chunk: the max-pass of
chunks 0/1 rides inside the DMA-bound projection phase, chunk qc+2's
max-pass matmuls are emitted before attention(qc) (so the DVE reduce
burst overlaps attention PE work and the row-64 staging is ready early),
and o_proj of chunk qc-1 is emitted mid-way through attention(qc).
"""
import sys

sys.path.insert(0, "/opt/trn_rl_repo")

from contextlib import ExitStack

import numpy as np

import concourse.bass as bass
import concourse.mybir as mybir
import concourse.tile as tile
from concourse import bacc
from concourse.bass_utils import run_bass_kernel_spmd
from concourse.masks import make_identity

# problem shapes (hardcoded per contract)
N = 2048
D = 1024
E = 64
H = 16
N_CORES = 8
H_PER_CORE = H // N_CORES  # 2

QC = 512          # q-chunk (moving dim of S'/ctx matmuls)
NQ = N // QC      # 4
MB = 128          # m-block (partition dim of S'^T tiles)
NMB = N // MB     # 16
DCH = D // 128    # 8 d-chunks for projections

F32 = mybir.dt.float32
F32R = mybir.dt.float32r

# dtype config: the scores path is precision-critical (softmax amplifies
# score errors exponentially).  SPLIT_SCORES uses an exact hi/lo float32r
# decomposition (fp32 accuracy at f32r speed); ctx and o_proj tolerate
# f32r's ~1e-4 rel error directly.
SPLIT_SCORES = True
CTX_F32R = True
OPROJ_F32R = True

_CACHE = {}


def build_nc():
    nc = bacc.Bacc(None, target_bir_lowering=False, debug=False)

    # x^T and the qkv weights arrive hi/lo-split into float32r halves
    # (host-side RNE-11-bit rounding, which matches the hardware exactly;
    # 11-bit operands multiply exactly, so the 3-term split matmul is
    # fp32-accurate at full float32r PE rate)
    xh = nc.declare_dram_parameter("xh", [D, N], F32R, isOutput=False)
    xl = nc.declare_dram_parameter("xl", [D, N], F32R, isOutput=False)
    wq = nc.declare_dram_parameter("wq", [D, 256], F32R, isOutput=False)
    wk = nc.declare_dram_parameter("wk", [D, 256], F32R, isOutput=False)
    wv = nc.declare_dram_parameter("wv", [D, 256], F32R, isOutput=False)
    wo = nc.declare_dram_parameter("wo", [128, D],
                                   F32R if OPROJ_F32R else F32,
                                   isOutput=False)
    out = nc.declare_dram_parameter("out", [N, D], F32, isOutput=True)

    ctx_dt = F32R if CTX_F32R else F32
    oproj_dt = F32R if OPROJ_F32R else F32
    sc_dt = F32R if SPLIT_SCORES else F32

    with ExitStack() as ctx:
        tc = ctx.enter_context(tile.TileContext(nc))
        singles = ctx.enter_context(tc.tile_pool(name="singles", bufs=1))
        ps = ctx.enter_context(tc.tile_pool(name="ps", bufs=8, space="PSUM"))
        ex_pool = ctx.enter_context(tc.tile_pool(name="ex", bufs=5))
        bc_pool = ctx.enter_context(tc.tile_pool(name="bc", bufs=2))

        ident = singles.tile([128, 128], F32)
        make_identity(nc, ident)

        # long-lived SBUF tensors
        qT_ext = [singles.tile([65, N], sc_dt, tag=f"qT_ext{h}", name=f"qT_ext{h}")
                  for h in range(2)]
        kT_ext = [singles.tile([65, N], sc_dt, tag=f"kT_ext{h}", name=f"kT_ext{h}")
                  for h in range(2)]
        qTr = singles.tile([128, N], F32R, tag="qTr")   # hi parts, heads packed
        kTr = singles.tile([128, N], F32R, tag="kTr")
        if SPLIT_SCORES:
            # stacked cross-term operands: one K=128 matmul computes
            # kl@qh + kh@ql.  qx = [qh; ql], kx = [kl; kh] (per head).
            qx = [singles.tile([128, N], F32R, tag=f"qx{h}", name=f"qx{h}")
                  for h in range(2)]
            kx = [singles.tile([128, N], F32R, tag=f"kx{h}", name=f"kx{h}")
                  for h in range(2)]
        v_ext = [singles.tile([128, NMB, 65], ctx_dt, tag=f"v_ext{h}",
                              name=f"v_ext{h}") for h in range(2)]
        mneg = [singles.tile([128, NMB], F32, tag=f"mneg{h}", name=f"mneg{h}")
                for h in range(2)]
        ctxn = singles.tile([128, N], oproj_dt, tag="ctxn")
        wo_sb = singles.tile([128, D], oproj_dt, tag="wo_sb")

        # ------- phases 2-4: max pass / attention / o_proj, pipelined -------
        # mp_mms emits one m-chunk's worth of max-pass matmuls+reduces; the
        # staging (transpose + row-64 DMA) is emitted separately so the PE
        # never waits in-order on a reduce burst that hasn't had time to run.
        mp_m4 = {}
        mp_m4_1 = {}

        def mp_mms(qc, mc, m4_tiles):
            for qbl in range(QC // 128):
                qb = qc * (QC // 128) + qbl
                if mc == 0 and qbl == 0:
                    for h in range(2):
                        m4_tiles[h] = bc_pool.tile(
                            [128, QC // 128, NQ], F32, tag=f"m4_{h}",
                            name=f"m4_{h}")
                pts = []
                # adjacent emission of the two heads' matmuls -> they run
                # concurrently in disjoint PE row groups
                for h in range(2):
                    hs = slice(h * 64, (h + 1) * 64)
                    pt = ps.tile([128, QC], F32, tag="ps", name=f"mp{h}")
                    pts.append(pt)
                    nc.tensor.matmul(
                        pt,
                        qTr[hs, qb * 128:(qb + 1) * 128],
                        kTr[hs, mc * QC:(mc + 1) * QC],
                        start=True,
                        stop=True,
                        tile_position=(h * 64, 0),
                    )
                for h in range(2):
                    nc.vector.reduce_max(
                        out=m4_tiles[h][:, qbl, mc:mc + 1], in_=pts[h],
                        axis=mybir.AxisListType.X,
                    )

        def mp_finish(qc, m4_tiles):
            qsl = slice(qc * QC, (qc + 1) * QC)
            for h in range(2):
                # one 3D reduce combines all four q-blocks' partial maxes
                nc.vector.reduce_max(
                    out=mneg[h][:, qc * NQ:(qc + 1) * NQ],
                    in_=m4_tiles[h],
                    axis=mybir.AxisListType.X,
                    negate=True,
                )
            # stage this chunk's -max values into qT_ext row 64: transpose
            # [128, 4] -> [4, 128] (rounded to scores dtype), then the
            # partition-major DMA stream of [4, 128] is exactly [1, 512]
            for h in range(2):
                ptm = ps.tile([4, 128], F32, tag="ps", name="ptm")
                nc.tensor.transpose(
                    ptm, mneg[h][:, qc * NQ:(qc + 1) * NQ], ident
                )
                mt_sb = bc_pool.tile([4, 128], sc_dt, tag="mt_sb")
                nc.vector.tensor_copy(mt_sb, ptm)
                nc.sync.dma_start(out=qT_ext[h][64:65, qsl], in_=mt_sb)


        # ---------------- phase 1: projections ----------------
        with tc.tile_pool(name="ph1", bufs=1) as ph1:
            vT_sb = ph1.tile([128, N], F32, tag="vT_sb")
            ones_cols = ph1.tile([128, NMB, 1], F32)
            nc.vector.memset(ones_cols, 1.0)
            ones_row = ph1.tile([1, N], F32)
            nc.vector.memset(ones_row, 1.0)
            for h in range(2):
                # ones row of kT_ext (cast-copy; memset can't write f32r)
                nc.vector.tensor_copy(kT_ext[h][64:65, :], ones_row)
                # col 64 of each v_ext block = 1.0
                nc.vector.tensor_copy(v_ext[h][:, :, 64:65], ones_cols)

            # wo arrives host-rounded to f32r (identical to the device
            # cast, verified), so it DMAs straight into the f32r tile
            nc.sync.dma_start(out=wo_sb, in_=wo[:, :])

            w_sb = {}
            for name, w in (("q", wq), ("k", wk), ("v", wv)):
                w_sb[name] = ph1.tile([128, DCH, 256], F32R, tag=f"w_{name}",
                                      name=f"w_{name}")
            wq_r = wq.rearrange("(c p) e -> p c e", p=128)
            xh_r = xh.rearrange("(c p) n -> p c n", p=128)
            xl_r = xl.rearrange("(c p) n -> p c n", p=128)

            # stream x hi/lo per n-chunk of QCP, double-buffered
            QCP = 256
            NQP = N // QCP
            with tc.tile_pool(name="xs", bufs=2) as xs_pool:
                for nchunk in range(NQP):
                    sl = slice(nchunk * QCP, (nchunk + 1) * QCP)
                    xht = xs_pool.tile([128, DCH, QCP], F32R, tag="xht")
                    xlt = xs_pool.tile([128, DCH, QCP], F32R, tag="xlt")
                    if nchunk == 0:
                        # fine-grained first chunk, interleaved per c-slice:
                        # matmul c starts as soon as its wq/xh/xl slices land
                        for c in range(DCH):
                            nc.sync.dma_start(out=w_sb["q"][:, c, :],
                                              in_=wq_r[:, c, :])
                            nc.sync.dma_start(out=xht[:, c, :],
                                              in_=xh_r[:, c, sl])
                            nc.sync.dma_start(out=xlt[:, c, :],
                                              in_=xl_r[:, c, sl])
                        for nm, w in (("k", wk), ("v", wv)):
                            nc.sync.dma_start(
                                out=w_sb[nm],
                                in_=w.rearrange("(c p) e -> p c e", p=128))
                    else:
                        nc.sync.dma_start(out=xht, in_=xh_r[:, :, sl])
                        nc.sync.dma_start(out=xlt, in_=xl_r[:, :, sl])
                    for name in ("q", "k", "v"):
                        pt = ps.tile([128, QCP], F32, tag="ps")
                        nmm = 3 * DCH
                        i = 0
                        for c in range(DCH):
                            # exact split: xh@wh + xl@wh + xh@wl (weight cols
                            # 0:128 = hi both heads, 128:256 = lo)
                            for wsl, xt_ in ((slice(0, 128), xht),
                                             (slice(0, 128), xlt),
                                             (slice(128, 256), xht)):
                                nc.tensor.matmul(
                                    pt,
                                    w_sb[name][:, c, wsl],
                                    xt_[:, c, :],
                                    start=(i == 0),
                                    stop=(i == nmm - 1),
                                )
                                i += 1
                        if name == "v":
                            nc.scalar.copy(out=vT_sb[:, sl], in_=pt)
                        else:
                            dst_ext = qT_ext if name == "q" else kT_ext
                            dst_r = qTr if name == "q" else kTr
                            dst_x = qx if name == "q" else kx
                            hi_rows = (slice(0, 64) if name == "q"
                                       else slice(64, 128))
                            lo_rows = (slice(64, 128) if name == "q"
                                       else slice(0, 64))
                            if nchunk >= 4:
                                nc.scalar.copy(out=dst_r[:, sl], in_=pt)
                            else:
                                nc.vector.tensor_copy(dst_r[:, sl], pt)
                            for h in range(2):
                                hs = slice(h * 64, (h + 1) * 64)
                                # per-head hi copies: SBUF->SBUF from the
                                # rounded packed tensor, on idle GpSimd
                                nc.gpsimd.tensor_copy(
                                    dst_ext[h][0:64, sl], dst_r[hs, sl])
                                if SPLIT_SCORES:
                                    nc.gpsimd.tensor_copy(
                                        dst_x[h][hi_rows, sl], dst_r[hs, sl])
                                    # lo residual: fp32 psum - f32r hi, rounded
                                    nc.vector.tensor_sub(
                                        dst_x[h][lo_rows, sl],
                                        pt[hs, :], dst_r[hs, sl])
                    # chunks 0 and 1 of the max pass ride along with
                    # phase 1 (filling DMA-bound PE idle): chunk 0's m-chunk
                    # mc needs qTr block 0 + kTr chunk mc (two 256-wide
                    # phase-1 chunks); chunk 1 additionally needs qTr
                    # blocks 4-7 (ready after phase-1 chunk 3)
                    if nchunk % 2 == 1:
                        mp_mms(0, nchunk // 2, mp_m4)
                    if nchunk == 3:
                        mp_mms(1, 0, mp_m4_1)
                        mp_mms(1, 1, mp_m4_1)
                    elif nchunk == 5:
                        mp_mms(1, 2, mp_m4_1)
                    elif nchunk == 7:
                        mp_mms(1, 3, mp_m4_1)

            # v_ext: transpose vT [64, N] -> v [m, e] blocks of [128, 64].
            # head-inner order: the two heads' transposes use disjoint PE
            # row groups (0-63 / 64-127), so adjacent emission lets them
            # run concurrently in the array on hardware
            for nb in range(NMB):
                for h in range(2):
                    ptt = ps.tile([128, 64], F32, tag="ps")
                    nc.tensor.transpose(
                        ptt,
                        vT_sb[h * 64:(h + 1) * 64, nb * 128:(nb + 1) * 128],
                        ident[h * 64:(h + 1) * 64, h * 64:(h + 1) * 64],
                    )
                    nc.scalar.copy(out=v_ext[h][:, nb, 0:64], in_=ptt)

        def attention_chunk(qc, seq_heads=False, mid_cb=None):
            qsl = slice(qc * QC, (qc + 1) * QC)
            ctx_ps = [ps.tile([65, QC], F32, tag="ps", name=f"ctx_ps{h}")
                      for h in range(2)]
            heads_order = ([(mb, h) for mb in range(NMB) for h in range(2)]
                           if not seq_heads else
                           [(mb, h) for h in range(2) for mb in range(NMB)])

            def emit_m1_tail(sp, mb, h):
                # the only matmul that reads row 64 (the staged -max row);
                # lagging it one m-block behind M2/M3 hides the staging
                # DMA latency at chunk entry
                nc.tensor.matmul(
                    sp, kT_ext[h][:, mb * 128:(mb + 1) * 128],
                    qT_ext[h][:, qsl],
                    start=False, stop=True,
                )
                et = ex_pool.tile([128, QC], ctx_dt, tag="et", name="et")
                nc.scalar.activation(
                    out=et, in_=sp,
                    func=mybir.ActivationFunctionType.Exp, scale=0.125,
                )
                nc.tensor.matmul(
                    ctx_ps[h], v_ext[h][:, mb, :], et,
                    start=(mb == 0), stop=(mb == NMB - 1),
                )

            lagged = []
            for it, (mb, h) in enumerate(heads_order):
                if it == 12 and mid_cb is not None:
                    mid_cb()
                msl = slice(mb * 128, (mb + 1) * 128)
                sp = ps.tile([128, QC], F32, tag="ps", name=f"sp{h}")
                # stacked cross terms first (no row-64 dependency):
                # one K=128 matmul = kl@qh + kh@ql
                nc.tensor.matmul(
                    sp, kx[h][:, msl], qx[h][:, qsl],
                    start=True, stop=False,
                )
                lagged.append((sp, mb, h))
                if len(lagged) > 1:
                    emit_m1_tail(*lagged.pop(0))
                if seq_heads and mb == NMB - 1:
                    while lagged:
                        emit_m1_tail(*lagged.pop(0))
                    norm_head(qc, h, ctx_ps)
            while lagged:
                emit_m1_tail(*lagged.pop(0))
            return ctx_ps

        def norm_head(qc, h, ctx_ps):
            qsl = slice(qc * QC, (qc + 1) * QC)
            # normalize: 1/Z broadcast over partitions on idle GpSimd
            rz = bc_pool.tile([1, QC], F32, tag="rz")
            nc.vector.reciprocal(out=rz, in_=ctx_ps[h][64:65, :])
            bc_sb = bc_pool.tile([64, QC], F32, tag="bc_sb")
            nc.gpsimd.partition_broadcast(bc_sb, rz)
            nc.vector.tensor_mul(
                ctxn[h * 64:(h + 1) * 64, qsl], ctx_ps[h][0:64, :], bc_sb
            )

        def norm_chunk(qc, ctx_ps, norm_done=False):
            if not norm_done:
                for h in range(2):
                    norm_head(qc, h, ctx_ps)

        def oproj_chunk(qc, fine_dma=False):
            # o_proj for this q-chunk (both heads fused: K=128); the two
            # 512-wide psum results merge into one [128, 1024] SBUF tile so
            # each n-block is a single contiguous output DMA.  For the final
            # chunk, per-half DMAs overlap the drain with the last copies.
            for nb in range(QC // 128):
                n0 = qc * QC + nb * 128
                po_sb = ex_pool.tile([128, D], F32, tag="po_sb", bufs=2)
                for dc in range(D // QC):
                    po = ps.tile([128, QC], F32, tag="ps", name="po")
                    nc.tensor.matmul(
                        po,
                        ctxn[:, n0:n0 + 128],
                        wo_sb[:, dc * QC:(dc + 1) * QC],
                        start=True,
                        stop=True,
                    )
                    nc.vector.tensor_copy(
                        po_sb[:, dc * QC:(dc + 1) * QC], po)
                    if fine_dma:
                        nc.sync.dma_start(
                            out=out[n0:n0 + 128, dc * QC:(dc + 1) * QC],
                            in_=po_sb[:, dc * QC:(dc + 1) * QC])
                if not fine_dma:
                    nc.sync.dma_start(out=out[n0:n0 + 128, :], in_=po_sb)

        # pipeline with 2-chunk max-pass lookahead: chunk 0's matmuls were
        # hoisted into phase 1; chunk qc+2's matmuls are emitted before
        # attention(qc) so chunk qc+1's reduces+staging are long done when
        # attention(qc+1)'s first score matmul reads row 64.  The last chunk
        # runs its heads sequentially so head 0's normalize chain overlaps
        # head 1's attention, shortening the drain tail.
        mp_finish(0, mp_m4)
        m4_next = mp_m4_1
        prev_oproj = None
        for qc in range(NQ):
            m4_next2 = {}
            if qc + 2 < NQ:
                for mc in range(NQ):
                    mp_mms(qc + 2, mc, m4_next2)
            seq = qc == NQ - 1
            # the previous chunk's o_proj is emitted mid-way through this
            # chunk's attention, by which point its normalize chain is done
            po = prev_oproj
            mid = (lambda: oproj_chunk(po)) if po is not None else None
            ctx_ps = attention_chunk(qc, seq_heads=seq, mid_cb=mid)
            if qc + 1 < NQ:
                mp_finish(qc + 1, m4_next)
            m4_next = m4_next2
            norm_chunk(qc, ctx_ps, norm_done=seq)
            prev_oproj = qc
        oproj_chunk(prev_oproj, fine_dma=True)

    nc.compile()
    return nc


def _round11(x):
    # round-to-nearest-even to 11 explicit mantissa bits — exactly the
    # hardware's float32r operand rounding (verified on device)
    u = np.ascontiguousarray(x, dtype=np.float32).view(np.uint32)
    shift = 23 - 11
    add = np.uint32((1 << (shift - 1)) - 1)
    lsb = (u >> np.uint32(shift)) & np.uint32(1)
    mask = np.uint32(~((1 << shift) - 1) & 0xFFFFFFFF)
    return ((u + add + lsb) & mask).view(np.float32)


def _split11(x):
    hi = _round11(x)
    lo = _round11(x.astype(np.float32) - hi)
    return hi, lo


def kernel(x, q_proj, k_proj, v_proj, o_proj):
    if "nc" not in _CACHE:
        _CACHE["nc"] = build_nc()
    nc = _CACHE["nc"]

    xT = np.ascontiguousarray(x.T.astype(np.float32, copy=False))
    xh, xl = _split11(xT)
    in_maps = []
    for core in range(N_CORES):
        h0 = core * H_PER_CORE

        def wsplit(w):
            w2 = np.concatenate([w[h0], w[h0 + 1]], axis=1)  # [D, 128]
            wh, wl = _split11(w2)
            return np.ascontiguousarray(np.concatenate([wh, wl], axis=1))

        m = {
            "xh": xh,
            "xl": xl,
            "wq": wsplit(q_proj),
            "wk": wsplit(k_proj),
            "wv": wsplit(v_proj),
            "wo": (_round11(o_proj[h0 * 64:(h0 + 2) * 64, :])
                   if OPROJ_F32R else
                   np.ascontiguousarray(o_proj[h0 * 64:(h0 + 2) * 64, :])),
        }
        in_maps.append(m)

    try:
        res = run_bass_kernel_spmd(nc, in_maps, core_ids=list(range(N_CORES)))
    except Exception:
        # one retry: a fresh NRT session recovers transient device faults
        res = run_bass_kernel_spmd(nc, in_maps, core_ids=list(range(N_CORES)))
    _CACHE["last_results"] = res
    acc = np.zeros((N, D), dtype=np.float64)
    for core in range(N_CORES):
        acc += res.results[core]["out"].astype(np.float64)
    return acc.astype(np.float32)


if __name__ == "__main__":
    rng = np.random.default_rng(0)
    ins = {
        "x": rng.standard_normal((N, D), dtype=np.float32),
        "q_proj": rng.standard_normal((H, D, E), dtype=np.float32),
        "k_proj": rng.standard_normal((H, D, E), dtype=np.float32),
        "v_proj": rng.standard_normal((H, D, E), dtype=np.float32),
        "o_proj": rng.standard_normal((D, D), dtype=np.float32),
    }
    out = kernel(**ins)
    print("out", out.shape, out.dtype, np.abs(out).max())



# revision 4
# speedup vs baseline: 1.0114x; 1.0114x over previous
"""Multi-head attention (16 heads, N=2048, D=1024, E=64) on 8 Trainium2 cores.

Head-parallel sharding: core m handles heads (2m, 2m+1), computes its two
heads' attention contexts and a partial o_proj (rows 128m:128m+128 of the
row-sharded o_proj); the host sums the 8 partial fp32 outputs in fp64.

Numerics (unchanged from the validated baseline): the softmax-score path
is fp32-accurate via an exact hi/lo float32r decomposition (host RNE-11
rounding == the PE's f32r operand rounding); v/ctx/o_proj tolerate plain
f32r. Scores per [m=128, q=512] tile are two f32r matmuls: a stacked
K=128 cross-term matmul [kl;kh]@[qh;ql] plus a K=65 kh_ext@qh_ext matmul
whose row 64 carries -rowmax(q) (computed by a separate hi-only max pass
in [q, m] layout, DVE-reduced); exp((S-c)/8) on ACT; ctx^T/Z accumulate
on PE via a ones-column in v_ext; 1/Z broadcast+mul normalizes.

Schedule (restructured around the cost model):
  - x is loaded ONCE as fp32 [128, N, 8] (c-interleaved, 8 DMAs of
    1MB, half the bytes of the old host-split hi/lo pair) and split
    into f32r hi/lo on device (DVE round-copy + GpSimd subtract — the
    device f32r convert equals the PE's operand rounding).
  - v projection is hi-only (1 matmul per d-chunk instead of 3): v is
    rounded to f32r downstream anyway, so the extra ~5e-4 relative
    error is far inside the 2e-2 budget.
  - no qTr/kTr intermediates: the hi parts live in known partition rows
    of qx/kx and the max pass reads them there.
  - PSUM is partitioned into dedicated pools (scores+oproj 3+1 banks,
    ctx 2, max-pass paired [128,2,512] tiles 2-4) so a lagging DVE
    reduce can never lock a bank the score matmuls need.
  - max-pass reduces are PAIRED (one XY-reduce over two m-chunks) and
    interleaved one unit per two attention tiles instead of bursting a
    whole chunk's worth onto DVE up front; the two proj-dependent
    pair-0 groups ride mid-projection, pair-1 groups + v_ext transposes
    fill the PE between projection end and attention(0).
  - GpSimd (no PSUM port) takes the SBUF-only copies/subs; o_proj
    psum->SBUF copies go to ACT; DVE keeps the reduces, psum-side
    copies and lo-residuals.
"""
import sys

sys.path.insert(0, "/opt/trn_rl_repo")

from contextlib import ExitStack

import numpy as np

import concourse.bass as bass
import concourse.mybir as mybir
import concourse.tile as tile
from concourse import bacc
from concourse.bass_utils import run_bass_kernel_spmd
from concourse.masks import make_identity

# problem shapes (hardcoded per contract)
N = 2048
D = 1024
E = 64
H = 16
N_CORES = 8
H_PER_CORE = H // N_CORES  # 2

QC = 512          # q-chunk (moving dim of S'/ctx matmuls)
NQ = N // QC      # 4
MB = 128          # m-block (partition dim of S'^T tiles)
NMB = N // MB     # 16
DCH = D // 128    # 8 d-chunks for projections
PC = 256          # projection n-chunk
NPC = N // PC     # 8

F32 = mybir.dt.float32
F32R = mybir.dt.float32r

_CACHE = {}


def build_nc():
    nc = bacc.Bacc(None, target_bir_lowering=False, debug=False)

    # x^T arrives once in fp32, c-interleaved: xf[p, n, c] = x[n, 128c+p].
    xf = nc.declare_dram_parameter("xf", [128, N, DCH], F32, isOutput=False)
    # q/k weights hi/lo-split on host (cols 0:128 = hi both heads,
    # 128:256 = lo); v weight is hi-only (f32r-level accuracy suffices).
    wq = nc.declare_dram_parameter("wq", [D, 256], F32R, isOutput=False)
    wk = nc.declare_dram_parameter("wk", [D, 256], F32R, isOutput=False)
    wv = nc.declare_dram_parameter("wv", [D, 128], F32R, isOutput=False)
    wo = nc.declare_dram_parameter("wo", [128, D], F32R, isOutput=False)
    out = nc.declare_dram_parameter("out", [N, D], F32, isOutput=True)

    with ExitStack() as ctx:
        tc = ctx.enter_context(tile.TileContext(nc))
        singles = ctx.enter_context(tc.tile_pool(name="singles", bufs=1))
        ex_pool = ctx.enter_context(tc.tile_pool(name="ex", bufs=3))
        bc_pool = ctx.enter_context(tc.tile_pool(name="bc", bufs=2))

        ident = singles.tile([128, 128], F32)
        make_identity(nc, ident)

        # long-lived SBUF tensors
        qT_ext = [singles.tile([65, N], F32R, tag=f"qT_ext{h}", name=f"qT_ext{h}")
                  for h in range(2)]
        kT_ext = [singles.tile([65, N], F32R, tag=f"kT_ext{h}", name=f"kT_ext{h}")
                  for h in range(2)]
        # stacked cross-term operands: one K=128 matmul computes
        # kl@qh + kh@ql.  qx = [qh; ql], kx = [kl; kh] (per head); the
        # hi rows double as the max-pass operands (no separate qTr/kTr).
        qx = [singles.tile([128, N], F32R, tag=f"qx{h}", name=f"qx{h}")
              for h in range(2)]
        kx = [singles.tile([128, N], F32R, tag=f"kx{h}", name=f"kx{h}")
              for h in range(2)]
        v_ext = [singles.tile([128, NMB, 65], F32R, tag=f"v_ext{h}",
                              name=f"v_ext{h}") for h in range(2)]
        vT_sb = singles.tile([128, N], F32, tag="vT_sb")
        mneg = [singles.tile([128, NQ], F32, tag=f"mneg{h}", name=f"mneg{h}")
                for h in range(2)]
        ctxn = singles.tile([128, N], F32R, tag="ctxn")
        wo_sb = singles.tile([128, D], F32R, tag="wo_sb")
        # per-chunk partial maxes: m4[qc][h][p, qbl, pair] (pair = 2 m-chunks)
        m4 = {}

        QHI, QLO = slice(0, 64), slice(64, 128)    # qx rows: [qh; ql]
        KLO, KHI = slice(0, 64), slice(64, 128)    # kx rows: [kl; kh]

        def mp_unit(pool, qc_t, qbl, pair, h):
            # one max-pass unit: two hi-only S[q, m] matmuls (512-wide
            # m-chunks 2*pair, 2*pair+1) into a paired psum tile, one
            # XY-reduce -> m4[qc_t][h][:, qbl, pair]
            qb = qc_t * (QC // 128) + qbl
            t = pool.tile([128, 2, QC], F32, tag="mp", name="mp")
            for j in range(2):
                mc = 2 * pair + j
                # hi operands via the ext tensors' rows 0:64 (partition-
                # aligned; qx/kx store hi at opposite halves)
                nc.tensor.matmul(
                    t[:, j, :],
                    qT_ext[h][0:64, qb * 128:(qb + 1) * 128],
                    kT_ext[h][0:64, mc * QC:(mc + 1) * QC],
                    start=True,
                    stop=True,
                )
            nc.vector.tensor_reduce(
                out=m4[qc_t][h][:, qbl, pair:pair + 1], in_=t,
                axis=mybir.AxisListType.XY, op=mybir.AluOpType.max,
            )

        def mp_alloc(qc_t):
            m4[qc_t] = [bc_pool.tile([128, NQ, 2], F32, tag=f"m4_{h}",
                                     name=f"m4_{h}") for h in range(2)]

        def mp_finish(qc, ptm_pool):
            qsl = slice(qc * QC, (qc + 1) * QC)
            for h in range(2):
                # combine the two pair-maxes per q-block, negated
                nc.vector.tensor_reduce(
                    out=mneg[h], in_=m4[qc][h],
                    axis=mybir.AxisListType.X, op=mybir.AluOpType.max,
                    negate=True,
                )
            # stage -max into qT_ext row 64: transpose [128, 4] -> [4, 128]
            # (rounded to f32r); the partition-major stream of [4, 128] is
            # exactly [1, 512]
            for h in range(2):
                ptm = ptm_pool.tile([4, 128], F32, tag="ptm", name="ptm",
                                    bufs=1)
                nc.tensor.transpose(ptm, mneg[h], ident)
                mt_sb = bc_pool.tile([4, 128], F32R, tag="mt_sb")
                nc.vector.tensor_copy(mt_sb, ptm)
                nc.sync.dma_start(out=qT_ext[h][64:65, qsl], in_=mt_sb)

        # ---------------- phase 1: projections ----------------
        with tc.tile_pool(name="mp1", bufs=2, space="PSUM") as mp1, \
             tc.tile_pool(name="xs", bufs=2) as xs_pool, \
             tc.tile_pool(name="ph1", bufs=1) as ph1:
            ones_cols = ph1.tile([128, NMB, 1], F32)
            nc.vector.memset(ones_cols, 1.0)
            ones_row = ph1.tile([1, N], F32)
            nc.vector.memset(ones_row, 1.0)
            for h in range(2):
                # ones row of kT_ext (cast-copy; memset can't write f32r)
                nc.vector.tensor_copy(kT_ext[h][64:65, :], ones_row)
                # col 64 of each v_ext block = 1.0
                nc.vector.tensor_copy(v_ext[h][:, :, 64:65], ones_cols)

            w_sb = {
                "q": ph1.tile([128, DCH, 256], F32R, tag="w_q", name="w_q"),
                "k": ph1.tile([128, DCH, 256], F32R, tag="w_k", name="w_k"),
                "v": ph1.tile([128, DCH, 128], F32R, tag="w_v", name="w_v"),
            }
            wq_r = wq.rearrange("(c p) e -> p c e", p=128)
            wk_r = wk.rearrange("(c p) e -> p c e", p=128)
            wv_r = wv.rearrange("(c p) e -> p c e", p=128)

            with tc.tile_pool(name="pp", bufs=1, space="PSUM") as pp:
                for nchunk in range(NPC):
                    sl = slice(nchunk * PC, (nchunk + 1) * PC)
                    xft = xs_pool.tile([128, PC, DCH], F32, tag="xft")
                    nc.sync.dma_start(out=xft, in_=xf[:, sl, :])
                    if nchunk == 0:
                        # weights land behind x chunk 0; wq per-c so
                        # matmul c starts as soon as its slice arrives
                        for c in range(DCH):
                            nc.sync.dma_start(out=w_sb["q"][:, c, :],
                                              in_=wq_r[:, c, :])
                        nc.sync.dma_start(out=w_sb["k"], in_=wk_r)
                        nc.sync.dma_start(out=w_sb["v"], in_=wv_r)
                        nc.sync.dma_start(out=wo_sb, in_=wo[:, :])
                    # device-side hi/lo split: the f32r round-copy equals
                    # the PE's operand rounding; lo = x - hi (the PE
                    # re-rounds the lo operand on read, matching the
                    # host-side round11(x - hi))
                    xht = xs_pool.tile([128, PC, DCH], F32R, tag="xht")
                    xlt = xs_pool.tile([128, PC, DCH], F32R, tag="xlt")
                    nc.vector.tensor_copy(xht, xft)
                    nc.gpsimd.tensor_sub(xlt, xft, xht)

                    for name in ("q", "k", "v"):
                        pt = pp.tile([128, PC], F32, tag=f"pt_{name}",
                                     name=f"pt_{name}")
                        terms = ((slice(0, 128), xht), (slice(0, 128), xlt),
                                 (slice(128, 256), xht))
                        if name == "v":
                            terms = ((slice(0, 128), xht),)
                        nmm = len(terms) * DCH
                        i = 0
                        for c in range(DCH):
                            for wsl, xt_ in terms:
                                nc.tensor.matmul(
                                    pt,
                                    w_sb[name][:, c, wsl],
                                    xt_[:, :, c],
                                    start=(i == 0),
                                    stop=(i == nmm - 1),
                                )
                                i += 1
                        if name == "v":
                            nc.scalar.copy(out=vT_sb[:, sl], in_=pt)
                            continue
                        dst_ext = qT_ext if name == "q" else kT_ext
                        dst_x = qx if name == "q" else kx
                        hi_rows = QHI if name == "q" else KHI
                        lo_rows = QLO if name == "q" else KLO
                        for h in range(2):
                            hs = slice(h * 64, (h + 1) * 64)
                            # hi: psum -> f32r (rounds); lo: fp32 psum -
                            # rounded hi (DVE; GpSimd has no PSUM port)
                            nc.vector.tensor_copy(
                                dst_x[h][hi_rows, sl], pt[hs, :])
                            nc.vector.tensor_sub(
                                dst_x[h][lo_rows, sl],
                                pt[hs, :], dst_x[h][hi_rows, sl])
                            # ext hi copy is SBUF->SBUF: GpSimd
                            nc.gpsimd.tensor_copy(
                                dst_ext[h][0:64, sl], dst_x[h][hi_rows, sl])

                    # max-pass pair-0 groups ride once kx[0:1024] exists
                    if nchunk == 3:
                        mp_alloc(0)
                        mp_alloc(1)
                        for qbl in range(4):
                            for h in range(2):
                                mp_unit(mp1, 0, qbl, 0, h)
                    elif nchunk == 4:
                        for qbl in range(4):
                            for h in range(2):
                                mp_unit(mp1, 1, qbl, 0, h)

            # post-proj: pair-1 groups (kx now complete) with the v_ext
            # transposes as PE filler so the chunk-0 staging chain (DVE
            # reduces -> finish -> row-64 DMA) completes before
            # attention(0)'s first ext matmul reads row 64
            with tc.tile_pool(name="pv", bufs=2, space="PSUM") as pv:
                for qbl in range(4):
                    for h in range(2):
                        mp_unit(mp1, 0, qbl, 1, h)
                mp_finish(0, pv)
                for qbl in range(4):
                    for h in range(2):
                        mp_unit(mp1, 1, qbl, 1, h)
                # v_ext: transpose vT [64, N] -> v [m, e] blocks [128, 64]
                for nb in range(NMB):
                    for h in range(2):
                        ptt = pv.tile([128, 64], F32, tag="ptt", name="ptt")
                        nc.tensor.transpose(
                            ptt,
                            vT_sb[h * 64:(h + 1) * 64,
                                  nb * 128:(nb + 1) * 128],
                            ident[h * 64:(h + 1) * 64, h * 64:(h + 1) * 64],
                        )
                        nc.scalar.copy(out=v_ext[h][:, nb, 0:64], in_=ptt)
                mp_finish(1, pv)

        # ---------------- phase 2: attention chunks ----------------
        sp_ps = ctx.enter_context(tc.tile_pool(name="sp", bufs=3, space="PSUM"))
        ctx_pool = ctx.enter_context(tc.tile_pool(name="cx", bufs=1, space="PSUM"))
        mp2 = ctx.enter_context(tc.tile_pool(name="mp2", bufs=1, space="PSUM"))

        def norm_head(qc, h, ctx_ps):
            qsl = slice(qc * QC, (qc + 1) * QC)
            rz = bc_pool.tile([1, QC], F32, tag="rz")
            nc.vector.reciprocal(out=rz, in_=ctx_ps[h][64:65, :])
            bc_sb = bc_pool.tile([64, QC], F32, tag="bc_sb")
            nc.gpsimd.partition_broadcast(bc_sb, rz)
            nc.vector.tensor_mul(
                ctxn[h * 64:(h + 1) * 64, qsl], ctx_ps[h][0:64, :], bc_sb
            )

        def oproj_block(qc, nb, fine_dma=False):
            # o_proj for one 128-row n-block (both heads fused: K=128);
            # the two 512-wide psum halves merge into one [128, 1024]
            # SBUF tile so the block is a single contiguous output DMA
            n0 = qc * QC + nb * 128
            po_sb = ex_pool.tile([128, D], F32, tag="po_sb", bufs=1)
            for dc in range(D // QC):
                po = sp_ps.tile([128, QC], F32, tag="sp", name="po")
                nc.tensor.matmul(
                    po,
                    ctxn[:, n0:n0 + 128],
                    wo_sb[:, dc * QC:(dc + 1) * QC],
                    start=True,
                    stop=True,
                )
                nc.scalar.copy(out=po_sb[:, dc * QC:(dc + 1) * QC], in_=po)
                if fine_dma:
                    nc.sync.dma_start(
                        out=out[n0:n0 + 128, dc * QC:(dc + 1) * QC],
                        in_=po_sb[:, dc * QC:(dc + 1) * QC])
            if not fine_dma:
                nc.sync.dma_start(out=out[n0:n0 + 128, :], in_=po_sb)

        def attention_chunk(qc, seq_heads=False):
            qsl = slice(qc * QC, (qc + 1) * QC)
            ctx_ps = [ctx_pool.tile([65, QC], F32, tag=f"ctx{h}",
                                    name=f"ctx_ps{h}") for h in range(2)]
            if not seq_heads:
                # h0's ctx matmuls lead so h1's norm (emitted last at the
                # previous chunk's end) has time to free its psum bank
                heads_order = ([(mb, 0) for mb in range(4)]
                               + [(mb, 1) for mb in range(4)]
                               + [(mb, h) for mb in range(4, NMB)
                                  for h in range(2)])
            else:
                heads_order = [(mb, h) for h in range(2) for mb in range(NMB)]
            started = {0: False, 1: False}

            # interleave schedules: one mp(qc+2) unit per 2 tiles, one
            # oproj(qc-1) block per 4 tiles starting mid-chunk
            mp_sched = {}
            if qc + 2 < NQ:
                mp_alloc(qc + 2)
                units = [(qbl, pair, h) for qbl in range(4)
                         for pair in range(2) for h in range(2)]
                for i, u in enumerate(units):
                    mp_sched[2 * i] = u
            po_sched = {}
            if qc > 0:
                for nb in range(4):
                    po_sched[10 + 4 * nb] = nb

            def emit_m1_tail(sp, mb, h):
                # the only matmul that reads row 64 (the staged -max row);
                # lagging it one tile behind the cross matmul hides the
                # staging DMA latency at chunk entry
                nc.tensor.matmul(
                    sp, kT_ext[h][:, mb * 128:(mb + 1) * 128],
                    qT_ext[h][:, qsl],
                    start=False, stop=True,
                )
                et = ex_pool.tile([128, QC], F32R, tag="et", name="et")
                nc.scalar.activation(
                    out=et, in_=sp,
                    func=mybir.ActivationFunctionType.Exp, scale=0.125,
                )
                nc.tensor.matmul(
                    ctx_ps[h], v_ext[h][:, mb, :], et,
                    start=not started[h], stop=(mb == NMB - 1),
                )
                started[h] = True

            lagged = []
            for it, (mb, h) in enumerate(heads_order):
                if it in mp_sched:
                    mp_unit(mp2, qc + 2, *mp_sched[it])
                if it in po_sched:
                    oproj_block(qc - 1, po_sched[it])
                msl = slice(mb * 128, (mb + 1) * 128)
                sp = sp_ps.tile([128, QC], F32, tag="sp", name=f"sp{h}")
                # stacked cross terms first (no row-64 dependency):
                # one K=128 matmul = kl@qh + kh@ql
                nc.tensor.matmul(
                    sp, kx[h][:, msl], qx[h][:, qsl],
                    start=True, stop=False,
                )
                lagged.append((sp, mb, h))
                if len(lagged) > 1:
                    emit_m1_tail(*lagged.pop(0))
                if seq_heads and mb == NMB - 1:
                    while lagged:
                        emit_m1_tail(*lagged.pop(0))
                    norm_head(qc, h, ctx_ps)
            while lagged:
                emit_m1_tail(*lagged.pop(0))
            return ctx_ps

        for qc in range(NQ):
            seq = qc == NQ - 1
            ctx_ps = attention_chunk(qc, seq_heads=seq)
            if qc + 1 < NQ:
                mp_finish(qc + 1, sp_ps)
            if not seq:
                for h in range(2):
                    norm_head(qc, h, ctx_ps)
        for nb in range(4):
            oproj_block(NQ - 1, nb, fine_dma=True)

    nc.compile()
    return nc


def _round11(x):
    # round-to-nearest-even to 11 explicit mantissa bits — exactly the
    # hardware's float32r operand rounding (verified on device)
    u = np.ascontiguousarray(x, dtype=np.float32).view(np.uint32)
    shift = 23 - 11
    add = np.uint32((1 << (shift - 1)) - 1)
    lsb = (u >> np.uint32(shift)) & np.uint32(1)
    mask = np.uint32(~((1 << shift) - 1) & 0xFFFFFFFF)
    return ((u + add + lsb) & mask).view(np.float32)


def _split11(x):
    hi = _round11(x)
    lo = _round11(x.astype(np.float32) - hi)
    return hi, lo


def kernel(x, q_proj, k_proj, v_proj, o_proj):
    if "nc" not in _CACHE:
        _CACHE["nc"] = build_nc()
    nc = _CACHE["nc"]

    # xf[p, n, c] = x[n, 128c+p]
    xf = np.ascontiguousarray(
        x.astype(np.float32, copy=False).reshape(N, DCH, 128).transpose(2, 0, 1)
    )
    in_maps = []
    for core in range(N_CORES):
        h0 = core * H_PER_CORE

        def wsplit(w):
            w2 = np.concatenate([w[h0], w[h0 + 1]], axis=1)  # [D, 128]
            wh, wl = _split11(w2)
            return np.ascontiguousarray(np.concatenate([wh, wl], axis=1))

        m = {
            "xf": xf,
            "wq": wsplit(q_proj),
            "wk": wsplit(k_proj),
            "wv": _round11(np.concatenate([v_proj[h0], v_proj[h0 + 1]],
                                          axis=1)),
            "wo": _round11(o_proj[h0 * 64:(h0 + 2) * 64, :]),
        }
        in_maps.append(m)

    try:
        res = run_bass_kernel_spmd(nc, in_maps, core_ids=list(range(N_CORES)))
    except Exception:
        # one retry: a fresh NRT session recovers transient device faults
        res = run_bass_kernel_spmd(nc, in_maps, core_ids=list(range(N_CORES)))
    _CACHE["last_results"] = res
    acc = np.zeros((N, D), dtype=np.float64)
    for core in range(N_CORES):
        acc += res.results[core]["out"].astype(np.float64)
    return acc.astype(np.float32)


if __name__ == "__main__":
    rng = np.random.default_rng(0)
    ins = {
        "x": rng.standard_normal((N, D), dtype=np.float32),
        "q_proj": rng.standard_normal((H, D, E), dtype=np.float32),
        "k_proj": rng.standard_normal((H, D, E), dtype=np.float32),
        "v_proj": rng.standard_normal((H, D, E), dtype=np.float32),
        "o_proj": rng.standard_normal((D, D), dtype=np.float32),
    }
    out = kernel(**ins)
    print("out", out.shape, out.dtype, np.abs(out).max())


# revision 13
# speedup vs baseline: 1.0849x; 1.0727x over previous
"""Multi-head attention (16 heads, N=2048, D=1024, E=64) on 8 Trainium2 cores.

Head-parallel sharding: core m handles heads (2m, 2m+1), computes its two
heads' attention contexts and a partial o_proj (rows 128m:128m+128 of the
row-sharded o_proj); the host sums the 8 partial fp32 outputs in fp64.

Numerics (unchanged from the validated baseline): the softmax-score path
is fp32-accurate via an exact hi/lo float32r decomposition (host RNE-11
rounding == the PE's f32r operand rounding); v/ctx/o_proj tolerate plain
f32r. Scores per [m=128, q=512] tile are two f32r matmuls: a stacked
K=128 cross-term matmul [kl;kh]@[qh;ql] plus a K=65 kh_ext@qh_ext matmul
whose row 64 carries -rowmax(q) (computed by a separate hi-only max pass
in [q, m] layout, DVE-reduced); exp((S-c)/8) on ACT; ctx^T/Z accumulate
on PE via a ones-column in v_ext; 1/Z broadcast+mul normalizes.

Schedule (restructured around the cost model):
  - x is loaded ONCE as fp32 [128, N, 8] (c-interleaved, 8 DMAs of
    1MB, half the bytes of the old host-split hi/lo pair) and split
    into f32r hi/lo on device (DVE round-copy + GpSimd subtract — the
    device f32r convert equals the PE's operand rounding).
  - v projection is hi-only (1 matmul per d-chunk instead of 3): v is
    rounded to f32r downstream anyway, so the extra ~5e-4 relative
    error is far inside the 2e-2 budget.
  - no qTr/kTr intermediates: the hi parts live in known partition rows
    of qx/kx and the max pass reads them there.
  - PSUM is partitioned into dedicated pools (scores+oproj 3+1 banks,
    ctx 2, max-pass paired [128,2,512] tiles 2-4) so a lagging DVE
    reduce can never lock a bank the score matmuls need.
  - max-pass reduces are PAIRED (one XY-reduce over two m-chunks) and
    interleaved one unit per two attention tiles instead of bursting a
    whole chunk's worth onto DVE up front; the two proj-dependent
    pair-0 groups ride mid-projection, pair-1 groups + v_ext transposes
    fill the PE between projection end and attention(0).
  - GpSimd (no PSUM port) takes the SBUF-only copies/subs; o_proj
    psum->SBUF copies go to ACT; DVE keeps the reduces, psum-side
    copies and lo-residuals.
"""
import sys

sys.path.insert(0, "/opt/trn_rl_repo")

from contextlib import ExitStack

import numpy as np

import concourse.bass as bass
import concourse.mybir as mybir
import concourse.tile as tile
from concourse import bacc
from concourse.bass_utils import run_bass_kernel_spmd
from concourse.masks import make_identity

# problem shapes (hardcoded per contract)
N = 2048
D = 1024
E = 64
H = 16
N_CORES = 8
H_PER_CORE = H // N_CORES  # 2

QC = 512          # q-chunk (moving dim of S'/ctx matmuls)
NQ = N // QC      # 4
MB = 128          # m-block (partition dim of S'^T tiles)
NMB = N // MB     # 16
DCH = D // 128    # 8 d-chunks for projections
PC = 256          # projection n-chunk
NPC = N // PC     # 8

F32 = mybir.dt.float32
F32R = mybir.dt.float32r

_CACHE = {}


def build_nc():
    nc = bacc.Bacc(None, target_bir_lowering=False, debug=False)

    # x^T arrives once in fp32, c-interleaved: xf[p, n, c] = x[n, 128c+p].
    xf = nc.declare_dram_parameter("xf", [128, N, DCH], F32, isOutput=False)
    # q/k weights hi/lo-split on host (cols 0:128 = hi both heads,
    # 128:256 = lo); v weight is hi-only (f32r-level accuracy suffices).
    wq = nc.declare_dram_parameter("wq", [D, 256], F32R, isOutput=False)
    wk = nc.declare_dram_parameter("wk", [D, 256], F32R, isOutput=False)
    wv = nc.declare_dram_parameter("wv", [D, 128], F32R, isOutput=False)
    wo = nc.declare_dram_parameter("wo", [128, D], F32R, isOutput=False)
    out = nc.declare_dram_parameter("out", [N, D], F32, isOutput=True)

    with ExitStack() as ctx:
        tc = ctx.enter_context(tile.TileContext(nc))
        singles = ctx.enter_context(tc.tile_pool(name="singles", bufs=1))
        ex_pool = ctx.enter_context(tc.tile_pool(name="ex", bufs=3))
        bc_pool = ctx.enter_context(tc.tile_pool(name="bc", bufs=2))

        ident = singles.tile([128, 128], F32)
        make_identity(nc, ident)

        # long-lived SBUF tensors
        qT_ext = [singles.tile([65, N], F32R, tag=f"qT_ext{h}", name=f"qT_ext{h}")
                  for h in range(2)]
        kT_ext = [singles.tile([65, N], F32R, tag=f"kT_ext{h}", name=f"kT_ext{h}")
                  for h in range(2)]
        # stacked cross-term operands: one K=128 matmul computes
        # kl@qh + kh@ql.  qx = [qh; ql], kx = [kl; kh] (per head); the
        # hi rows double as the max-pass operands (no separate qTr/kTr).
        qx = [singles.tile([128, N], F32R, tag=f"qx{h}", name=f"qx{h}")
              for h in range(2)]
        kx = [singles.tile([128, N], F32R, tag=f"kx{h}", name=f"kx{h}")
              for h in range(2)]
        v_ext = [singles.tile([128, NMB, 65], F32R, tag=f"v_ext{h}",
                              name=f"v_ext{h}") for h in range(2)]
        mneg = [singles.tile([128, NQ], F32, tag=f"mneg{h}", name=f"mneg{h}")
                for h in range(2)]
        ctxn = singles.tile([128, N], F32R, tag="ctxn")
        wo_sb = singles.tile([128, D], F32R, tag="wo_sb")
        # per-chunk partial maxes: m4[qc][h][p, qbl, pair] (pair = 2 m-chunks)
        m4 = {}

        QHI, QLO = slice(0, 64), slice(64, 128)    # qx rows: [qh; ql]
        KLO, KHI = slice(0, 64), slice(64, 128)    # kx rows: [kl; kh]

        def mp_unit(pool, qc_t, qbl, mcs, comp, h):
            # one max-pass unit: len(mcs) hi-only S[q, m] matmuls
            # (512-wide m-chunks) into one psum tile, one XY-reduce ->
            # m4[qc_t][h][:, qbl, comp].  Hi operands come from the ext
            # tensors' rows 0:64 (partition-aligned; qx/kx store hi at
            # opposite halves).
            qb = qc_t * (QC // 128) + qbl
            t = pool.tile([128, 2, QC], F32, tag="mp", name="mp")
            for j, mc in enumerate(mcs):
                nc.tensor.matmul(
                    t[:, j, :],
                    qT_ext[h][0:64, qb * 128:(qb + 1) * 128],
                    kT_ext[h][0:64, mc * QC:(mc + 1) * QC],
                    start=True,
                    stop=True,
                )
            nc.vector.tensor_reduce(
                out=m4[qc_t][h][:, qbl, comp:comp + 1],
                in_=t[:, 0:len(mcs), :],
                axis=mybir.AxisListType.XY, op=mybir.AluOpType.max,
            )

        def mp_alloc(qc_t, ncomp):
            m4[qc_t] = [bc_pool.tile([128, NQ, ncomp], F32,
                                     tag=f"m4_{h}_{ncomp}", name=f"m4_{h}")
                        for h in range(2)]

        def mp_finish(qc, ptm_pool, ptm_tag, ptm_shape):
            qsl = slice(qc * QC, (qc + 1) * QC)
            for h in range(2):
                # combine the two pair-maxes per q-block, negated
                nc.vector.tensor_reduce(
                    out=mneg[h], in_=m4[qc][h],
                    axis=mybir.AxisListType.X, op=mybir.AluOpType.max,
                    negate=True,
                )
            # stage -max into qT_ext row 64: transpose [128, 4] -> [4, 128]
            # (rounded to f32r); the partition-major stream of [4, 128] is
            # exactly [1, 512].  The psum scratch borrows a slot of the
            # caller's pool via its standard tag (same slot bytes).
            for h in range(2):
                ptm = ptm_pool.tile(ptm_shape, F32, tag=ptm_tag, name="ptm",
                                    space="PSUM")
                nc.tensor.transpose(ptm[0:4, 0:128], mneg[h], ident)
                mt_sb = bc_pool.tile([4, 128], F32R, tag="mt_sb")
                nc.vector.tensor_copy(mt_sb, ptm[0:4, 0:128])
                nc.sync.dma_start(out=qT_ext[h][64:65, qsl], in_=mt_sb)

        # ---------------- phase 1: projections ----------------
        with tc.tile_pool(name="mp1", bufs=2, space="PSUM") as mp1, \
             tc.tile_pool(name="xs", bufs=2) as xs_pool, \
             tc.tile_pool(name="ph1", bufs=1) as ph1:
            ones_cols = ph1.tile([128, NMB, 1], F32)
            nc.vector.memset(ones_cols, 1.0)
            ones_row = ph1.tile([1, N], F32)
            nc.vector.memset(ones_row, 1.0)
            for h in range(2):
                # ones row of kT_ext (cast-copy; memset can't write f32r)
                nc.vector.tensor_copy(kT_ext[h][64:65, :], ones_row)
                # col 64 of each v_ext block = 1.0
                nc.vector.tensor_copy(v_ext[h][:, :, 64:65], ones_cols)

            w_sb = {
                "q": ph1.tile([128, DCH, 256], F32R, tag="w_q", name="w_q"),
                "k": ph1.tile([128, DCH, 256], F32R, tag="w_k", name="w_k"),
                "v": ph1.tile([128, DCH, 128], F32R, tag="w_v", name="w_v"),
            }
            wq_r = wq.rearrange("(c p) e -> p c e", p=128)
            wk_r = wk.rearrange("(c p) e -> p c e", p=128)
            wv_r = wv.rearrange("(c p) e -> p c e", p=128)

            with tc.tile_pool(name="pp", bufs=1, space="PSUM") as pp:
                for nchunk in range(NPC):
                    sl = slice(nchunk * PC, (nchunk + 1) * PC)
                    xft = xs_pool.tile([128, PC, DCH], F32, tag="xft")
                    nc.sync.dma_start(out=xft, in_=xf[:, sl, :])
                    if nchunk == 0:
                        # weights land behind x chunk 0; wq per-c so
                        # matmul c starts as soon as its slice arrives
                        for c in range(DCH):
                            nc.sync.dma_start(out=w_sb["q"][:, c, :],
                                              in_=wq_r[:, c, :])
                        nc.sync.dma_start(out=w_sb["k"], in_=wk_r)
                        nc.sync.dma_start(out=w_sb["v"], in_=wv_r)
                        nc.sync.dma_start(out=wo_sb, in_=wo[:, :])
                    # device-side hi/lo split: the f32r round-copy equals
                    # the PE's operand rounding; lo = x - hi (the PE
                    # re-rounds the lo operand on read, matching the
                    # host-side round11(x - hi))
                    xht = xs_pool.tile([128, PC, DCH], F32R, tag="xht")
                    xlt = xs_pool.tile([128, PC, DCH], F32R, tag="xlt")
                    nc.vector.tensor_copy(xht, xft)
                    nc.vector.tensor_sub(xlt, xft, xht)

                    for name in ("q", "k", "v"):
                        pt = pp.tile([128, PC], F32, tag=f"pt_{name}",
                                     name=f"pt_{name}",
                                     bufs=2 if name == "v" else 1)
                        terms = ((slice(0, 128), xht), (slice(0, 128), xlt),
                                 (slice(128, 256), xht))
                        if name == "v":
                            terms = ((slice(0, 128), xht),)
                        nmm = len(terms) * DCH
                        i = 0
                        for c in range(DCH):
                            for wsl, xt_ in terms:
                                nc.tensor.matmul(
                                    pt,
                                    w_sb[name][:, c, wsl],
                                    xt_[:, :, c],
                                    start=(i == 0),
                                    stop=(i == nmm - 1),
                                )
                                i += 1
                        if name == "v":
                            vT_c = xs_pool.tile([128, PC], F32, tag="vT_c")
                            nc.scalar.copy(out=vT_c, in_=pt)
                            # v_ext: transpose vT [64, 128-block] ->
                            # v [m, e] blocks [128, 64], inline per chunk
                            # (psum scratch reuses the pt_v slot bytes)
                            for nb2 in range(PC // 128):
                                mb = nchunk * (PC // 128) + nb2
                                for h in range(2):
                                    ptt = pp.tile([128, PC], F32,
                                                  tag="pt_v", name="ptt",
                                                  bufs=2)
                                    nc.tensor.transpose(
                                        ptt[:, 0:64],
                                        vT_c[h * 64:(h + 1) * 64,
                                             nb2 * 128:(nb2 + 1) * 128],
                                        ident[h * 64:(h + 1) * 64,
                                              h * 64:(h + 1) * 64],
                                    )
                                    nc.scalar.copy(out=v_ext[h][:, mb, 0:64],
                                                   in_=ptt[:, 0:64])
                            continue
                        dst_ext = qT_ext if name == "q" else kT_ext
                        dst_x = qx if name == "q" else kx
                        hi_rows = QHI if name == "q" else KHI
                        lo_rows = QLO if name == "q" else KLO
                        for h in range(2):
                            hs = slice(h * 64, (h + 1) * 64)
                            # hi: psum -> f32r on ACT (rounds); lo: fp32
                            # psum - rounded hi on DVE (GpSimd has no
                            # PSUM port); ext hi copy SBUF->SBUF: GpSimd
                            nc.scalar.copy(
                                out=dst_x[h][hi_rows, sl], in_=pt[hs, :])
                            nc.vector.tensor_sub(
                                dst_x[h][lo_rows, sl],
                                pt[hs, :], dst_x[h][hi_rows, sl])
                            nc.gpsimd.tensor_copy(
                                dst_ext[h][0:64, sl], dst_x[h][hi_rows, sl])

                    # max-pass groups ride as their kT_ext m-ranges land:
                    # pair (0,1) after chunk 3, single mc=2 after chunk 5,
                    # mc=3 post-proj
                    if nchunk == 3:
                        mp_alloc(0, 3)
                        mp_alloc(1, 3)
                        for qbl in range(4):
                            for h in range(2):
                                mp_unit(mp1, 0, qbl, (0, 1), 0, h)
                    elif nchunk == 4:
                        for qbl in range(4):
                            for h in range(2):
                                mp_unit(mp1, 1, qbl, (0, 1), 0, h)
                    elif nchunk == 5:
                        for qbl in range(4):
                            for h in range(2):
                                mp_unit(mp1, 0, qbl, (2,), 1, h)
                    elif nchunk == 6:
                        for qbl in range(4):
                            for h in range(2):
                                mp_unit(mp1, 1, qbl, (2,), 1, h)

            # post-proj: only the mc=3 groups remain; chunk 0's staging
            # chain (DVE reduces -> finish -> row-64 DMA) is emitted
            # first so it completes before attention(0)'s first ext
            # matmul reads row 64
            with tc.tile_pool(name="pv", bufs=1, space="PSUM") as pv:
                for qbl in range(4):
                    for h in range(2):
                        mp_unit(mp1, 0, qbl, (3,), 2, h)
                mp_finish(0, pv, "ptm", [4, 128])
                for qbl in range(4):
                    for h in range(2):
                        mp_unit(mp1, 1, qbl, (3,), 2, h)
                mp_finish(1, pv, "ptm", [4, 128])

        # ---------------- phase 2: attention chunks ----------------
        sp_ps = ctx.enter_context(tc.tile_pool(name="sp", bufs=3, space="PSUM"))
        ctx_pool = ctx.enter_context(tc.tile_pool(name="cx", bufs=1, space="PSUM"))
        mp2 = ctx.enter_context(tc.tile_pool(name="mp2", bufs=1, space="PSUM"))

        def norm_head(qc, h, ctx_ps):
            qsl = slice(qc * QC, (qc + 1) * QC)
            rz = bc_pool.tile([1, QC], F32, tag="rz")
            nc.vector.reciprocal(out=rz, in_=ctx_ps[h][64:65, :])
            bc_sb = bc_pool.tile([64, QC], F32, tag="bc_sb")
            nc.gpsimd.partition_broadcast(bc_sb, rz)
            nc.vector.tensor_mul(
                ctxn[h * 64:(h + 1) * 64, qsl], ctx_ps[h][0:64, :], bc_sb
            )

        def oproj_block(qc, nb, fine_dma=False):
            # o_proj for one 128-row n-block (both heads fused: K=128);
            # the two 512-wide psum halves merge into one [128, 1024]
            # SBUF tile so the block is a single contiguous output DMA
            n0 = qc * QC + nb * 128
            po_sb = ex_pool.tile([128, D], F32, tag="po_sb", bufs=1)
            for dc in range(D // QC):
                po = sp_ps.tile([128, QC], F32, tag="po", name="po", bufs=1)
                nc.tensor.matmul(
                    po,
                    ctxn[:, n0:n0 + 128],
                    wo_sb[:, dc * QC:(dc + 1) * QC],
                    start=True,
                    stop=True,
                )
                nc.scalar.copy(out=po_sb[:, dc * QC:(dc + 1) * QC], in_=po)
                if fine_dma:
                    nc.sync.dma_start(
                        out=out[n0:n0 + 128, dc * QC:(dc + 1) * QC],
                        in_=po_sb[:, dc * QC:(dc + 1) * QC])
            if not fine_dma:
                nc.sync.dma_start(out=out[n0:n0 + 128, :], in_=po_sb)

        def attention_chunk(qc, seq_heads=False):
            qsl = slice(qc * QC, (qc + 1) * QC)
            ctx_ps = [ctx_pool.tile([65, QC], F32, tag=f"ctx{h}",
                                    name=f"ctx_ps{h}") for h in range(2)]
            if not seq_heads:
                # h0's ctx matmuls lead so h1's norm (emitted last at the
                # previous chunk's end) has time to free its psum bank
                heads_order = ([(mb, 0) for mb in range(4)]
                               + [(mb, 1) for mb in range(4)]
                               + [(mb, h) for mb in range(4, NMB)
                                  for h in range(2)])
            else:
                heads_order = [(mb, h) for h in range(2) for mb in range(NMB)]
            started = {0: False, 1: False}

            # interleave schedules: one mp(qc+2) unit per 2 tiles, one
            # oproj(qc-1) block per 4 tiles starting mid-chunk
            mp_sched = {}
            if qc + 2 < NQ:
                mp_alloc(qc + 2, 2)
                units = [(qbl, (2 * pair, 2 * pair + 1), pair, h)
                         for qbl in range(4)
                         for pair in range(2) for h in range(2)]
                for i, u in enumerate(units):
                    mp_sched[2 * i] = u
            po_sched = {}
            if qc > 0:
                for nb in range(4):
                    po_sched[10 + 4 * nb] = nb

            def emit_m1_tail(sp, mb, h):
                # the only matmul that reads row 64 (the staged -max row);
                # lagging it one tile behind the cross matmul hides the
                # staging DMA latency at chunk entry
                nc.tensor.matmul(
                    sp, kT_ext[h][:, mb * 128:(mb + 1) * 128],
                    qT_ext[h][:, qsl],
                    start=False, stop=True,
                )
                et = ex_pool.tile([128, QC], F32R, tag="et", name="et")
                nc.scalar.activation(
                    out=et, in_=sp,
                    func=mybir.ActivationFunctionType.Exp, scale=0.125,
                )
                nc.tensor.matmul(
                    ctx_ps[h], v_ext[h][:, mb, :], et,
                    start=not started[h], stop=(mb == NMB - 1),
                )
                started[h] = True

            lagged = []
            for it, (mb, h) in enumerate(heads_order):
                if it in mp_sched:
                    mp_unit(mp2, qc + 2, *mp_sched[it])
                if it in po_sched:
                    oproj_block(qc - 1, po_sched[it])
                msl = slice(mb * 128, (mb + 1) * 128)
                sp = sp_ps.tile([128, QC], F32, tag="sp", name=f"sp{h}")
                # stacked cross terms first (no row-64 dependency):
                # one K=128 matmul = kl@qh + kh@ql
                nc.tensor.matmul(
                    sp, kx[h][:, msl], qx[h][:, qsl],
                    start=True, stop=False,
                )
                lagged.append((sp, mb, h))
                if len(lagged) > 1:
                    emit_m1_tail(*lagged.pop(0))
                if seq_heads and mb == NMB - 1:
                    while lagged:
                        emit_m1_tail(*lagged.pop(0))
                    norm_head(qc, h, ctx_ps)
            while lagged:
                emit_m1_tail(*lagged.pop(0))
            return ctx_ps

        for qc in range(NQ):
            seq = qc == NQ - 1
            ctx_ps = attention_chunk(qc, seq_heads=seq)
            if qc + 1 < NQ:
                mp_finish(qc + 1, sp_ps, "sp", [128, QC])
            if not seq:
                for h in range(2):
                    norm_head(qc, h, ctx_ps)
        for nb in range(4):
            oproj_block(NQ - 1, nb, fine_dma=True)

    nc.compile()
    return nc


def _round11(x):
    # round-to-nearest-even to 11 explicit mantissa bits — exactly the
    # hardware's float32r operand rounding (verified on device)
    u = np.ascontiguousarray(x, dtype=np.float32).view(np.uint32)
    shift = 23 - 11
    add = np.uint32((1 << (shift - 1)) - 1)
    lsb = (u >> np.uint32(shift)) & np.uint32(1)
    mask = np.uint32(~((1 << shift) - 1) & 0xFFFFFFFF)
    return ((u + add + lsb) & mask).view(np.float32)


def _split11(x):
    hi = _round11(x)
    lo = _round11(x.astype(np.float32) - hi)
    return hi, lo


def kernel(x, q_proj, k_proj, v_proj, o_proj):
    if "nc" not in _CACHE:
        _CACHE["nc"] = build_nc()
    nc = _CACHE["nc"]

    # xf[p, n, c] = x[n, 128c+p]
    xf = np.ascontiguousarray(
        x.astype(np.float32, copy=False).reshape(N, DCH, 128).transpose(2, 0, 1)
    )
    in_maps = []
    for core in range(N_CORES):
        h0 = core * H_PER_CORE

        def wsplit(w):
            w2 = np.concatenate([w[h0], w[h0 + 1]], axis=1)  # [D, 128]
            wh, wl = _split11(w2)
            return np.ascontiguousarray(np.concatenate([wh, wl], axis=1))

        m = {
            "xf": xf,
            "wq": wsplit(q_proj),
            "wk": wsplit(k_proj),
            "wv": _round11(np.concatenate([v_proj[h0], v_proj[h0 + 1]],
                                          axis=1)),
            "wo": _round11(o_proj[h0 * 64:(h0 + 2) * 64, :]),
        }
        in_maps.append(m)

    try:
        res = run_bass_kernel_spmd(nc, in_maps, core_ids=list(range(N_CORES)))
    except Exception:
        # one retry: a fresh NRT session recovers transient device faults
        res = run_bass_kernel_spmd(nc, in_maps, core_ids=list(range(N_CORES)))
    _CACHE["last_results"] = res
    acc = np.zeros((N, D), dtype=np.float64)
    for core in range(N_CORES):
        acc += res.results[core]["out"].astype(np.float64)
    return acc.astype(np.float32)


if __name__ == "__main__":
    rng = np.random.default_rng(0)
    ins = {
        "x": rng.standard_normal((N, D), dtype=np.float32),
        "q_proj": rng.standard_normal((H, D, E), dtype=np.float32),
        "k_proj": rng.standard_normal((H, D, E), dtype=np.float32),
        "v_proj": rng.standard_normal((H, D, E), dtype=np.float32),
        "o_proj": rng.standard_normal((D, D), dtype=np.float32),
    }
    out = kernel(**ins)
    print("out", out.shape, out.dtype, np.abs(out).max())
